# revision 15
# baseline (speedup 1.0000x reference)
"""Trainium2 Bass kernel for the digit-conv model.

Math: y = relu(relu(conv3x3(x) @ W1 + b1) @ W2 + b2) @ W3 + b3.
The valid 3x3 conv is linear, so it folds into W1 on device:
feat = x @ A with A[u, q] sparse from conv_w, hence
W1eff = A @ W1 and y = mlp(x @ W1eff ...). The kernel computes
W1eff = A^T.T @ W1 on the tensor engine once (A^T is banded, so
all-zero blocks are statically skipped), then streams the batch
through the 3-layer MLP entirely as lhsT.T @ rhs matmuls with channels
on partitions and batch on the free dimension (no transposes needed:
x is supplied pre-transposed per shard, and every weight is already in
[K, M] layout).

Sharding: pure data parallel — batch split across 8 cores, weights
replicated. Host-side work is limited to layout (x transpose + shard +
bf16 cast, zero-padding, band extraction) and scattering the 9 conv
weights into the A^T matrix (no arithmetic).

All matmul operands are bf16: fp32 x (25.7 MB/core) would be DMA-bound
(~250 GB/s/core effective -> ~103 us, above the ~97 us tensor-engine
floor). PSUM accumulation is fp32; biases are applied in fp32 from
PSUM. (f32r was measured more accurate but cannot be mixed with bf16:
walrus inserts round-to-fp32r passes over DMA-loaded f32r tiles that
corrupt neighboring tiles in a mixed-dtype program.)

DMA instruction count is minimized (one multi-tile DMA per logical
group via 3D access patterns): each dma_start costs ~650 ns of serial
issue on its queue engine, which dominated startup in earlier
revisions. Constants go through the GpSimd (SWDGE) path so x loads
start issuing immediately on the Sync (HWDGE) path. A block of dummy
matmuls on a memset tile warms the PE clock-gate (HAM) before real
work arrives, and the first batch super-blocks are narrower so the
main loop starts after ~1.3 MB of x instead of ~3.2 MB.
"""

import ml_dtypes
import numpy as np

import concourse.tile as tile
from concourse import bacc, mybir
from concourse import bass_utils

N_CORES = 8
B = 65536
BC = B // N_CORES  # 8192 rows per core
U = 784            # input features (28*28)
Q = 676            # conv outputs (26*26)
QP = 768           # q padded to 6 full tiles of 128
H1, H2, H3 = 300, 100, 10
NB = 512           # batch columns per PSUM block (one bank of fp32)
SUP = 2048         # max batch columns per DMA super-block
SUP_WIDTHS = [512, 512, 1024, 2048, 2048, 2048]
assert sum(SUP_WIDTHS) == BC
KT = 112           # u-dim k-tile (784 = 7*112)
NKT = 7
MC = 100           # layer-1 output chunk (300 = 3*100)
NMC = 3
ABW = 336          # amat band width (3 u-chunks), fixed for all q-tiles

_prog_cache = {}


def _fold_bands():
    """Static block-sparsity of A^T [Q, U]: per 128-row q-tile, the nonzero
    columns lie in a band; returns per-tile (q0, p_real, c_lo, c_hi) with the
    band given in whole 112-wide u-chunks (at most 3 chunks wide)."""
    bands = []
    for qt in range(QP // 128):
        q0 = qt * 128
        p_real = min(128, Q - q0)
        i_lo = q0 // 26
        i_hi = (q0 + p_real - 1) // 26
        u_lo = 28 * i_lo
        u_hi = min(U, 28 * (i_hi + 2) + 28)   # exclusive upper bound
        c_lo = u_lo // KT
        c_hi = (u_hi + KT - 1) // KT          # exclusive chunk bound
        assert c_hi - c_lo <= ABW // KT
        bands.append((q0, p_real, c_lo, c_hi))
    return bands


def _build_program():
    f32 = mybir.dt.float32
    bf16 = mybir.dt.bfloat16
    relu = mybir.ActivationFunctionType.Relu
    alu_add = mybir.AluOpType.add
    alu_max = mybir.AluOpType.max

    nc = bacc.Bacc(
        "TRN2", target_bir_lowering=False, debug=False, num_devices=N_CORES
    )

    nqt = QP // 128
    xT_d = nc.dram_tensor("xT", [U, BC], bf16, kind="ExternalInput").ap()
    FW = ABW + H1  # 636: packed [amat band | w1] row width
    fold_d = nc.dram_tensor("fold", [QP, FW], bf16, kind="ExternalInput").ap()
    w2_d = nc.dram_tensor("w2", [H1, H2], bf16, kind="ExternalInput").ap()
    w3_d = nc.dram_tensor("w3", [H2, H3], bf16, kind="ExternalInput").ap()
    bias_d = nc.dram_tensor("bias", [MC, 5], f32, kind="ExternalInput").ap()
    yT_d = nc.dram_tensor("yT", [H3, BC], f32, kind="ExternalOutput").ap()

    bands = _fold_bands()

    with tile.TileContext(nc) as tc:
        with tc.tile_pool(name="const", bufs=1) as cpool, \
             tc.tile_pool(name="xp", bufs=4) as xpool, \
             tc.tile_pool(name="hp", bufs=2) as hpool, \
             tc.tile_pool(name="yp", bufs=2) as ypool, \
             tc.tile_pool(name="ps1", bufs=4, space="PSUM") as ps1p, \
             tc.tile_pool(name="ps2", bufs=2, space="PSUM") as ps2p, \
             tc.tile_pool(name="ps3", bufs=2, space="PSUM") as ps3p:

            # ---- HAM warmup: dummy matmuls on a memset tile so the PE
            # clock-gate releases before the real work arrives ----
            warm_sb = cpool.tile([128, 128], bf16)
            nc.vector.memset(warm_sb[:], 0.0)
            for wi in range(44):
                pw = ps2p.tile([128, 64], f32, tag="l2", name=f"pwarm_{wi}")
                nc.tensor.matmul(pw[:], warm_sb[:, :128], warm_sb[:, :64],
                                 start=True, stop=True)

            # ---- constants into SBUF (one merged DMA per group, on the
            # SWDGE path so they don't block x-load issue on HWDGE) ----
            fold_sb = cpool.tile([128, nqt * FW], bf16)
            nc.sync.dma_start(
                fold_sb[:].rearrange("p (q c) -> p q c", c=FW),
                fold_d.rearrange("(q p) c -> p q c", p=128),
            )
            w2_sb = cpool.tile([MC, NMC * H2], bf16)
            nc.gpsimd.dma_start(
                w2_sb[:].rearrange("p (k c) -> p k c", c=H2),
                w2_d.rearrange("(k p) c -> p k c", p=MC),
            )
            w3_sb = cpool.tile([H2, H3], bf16)
            nc.gpsimd.dma_start(w3_sb[:], w3_d)
            bias_sb = cpool.tile([MC, 5], f32)
            nc.gpsimd.dma_start(bias_sb[:], bias_d)

            # ---- fold the conv into W1: W1eff[u, c] = (A^T).T @ W1 ----
            # Only q-tiles whose band covers the u-chunk contribute; the
            # rest are all-zero blocks of the banded A^T and are skipped.
            # (fold PSUM shares the l1 slot group: same tag, bank-sized)
            w1eff_t = []
            for ut in range(NKT):
                parts = [qt for qt, (_, _, c_lo, c_hi) in enumerate(bands)
                         if c_lo <= ut < c_hi]
                assert parts
                pf = ps1p.tile([KT, NB], f32, tag="l1", name=f"pfold_{ut}")
                for idx, qt in enumerate(parts):
                    _, _, c_lo, _ = bands[qt]
                    off = qt * FW + (ut - c_lo) * KT
                    nc.tensor.matmul(
                        pf[:, :H1],
                        fold_sb[:, off:off + KT],
                        fold_sb[:, qt * FW + ABW:(qt + 1) * FW],
                        start=(idx == 0),
                        stop=(idx == len(parts) - 1),
                    )
                we = cpool.tile([KT, H1], bf16, name=f"w1eff_{ut}")
                nc.vector.tensor_copy(we[:], pf[:, :H1])
                w1eff_t.append(we)

            # ---- main pipeline over batch super-blocks ----
            sup_start = 0
            for sup, sw in enumerate(SUP_WIDTHS):
                xtile = xpool.tile([KT, NKT * sw], bf16, tag="x",
                                   name=f"xt_{sup}",
                                   padded_shape=[KT, NKT * SUP])
                nc.sync.dma_start(
                    xtile[:].rearrange("p (k c) -> p k c", c=sw),
                    xT_d[:, sup_start:sup_start + sw]
                    .rearrange("(k p) c -> p k c", p=KT),
                )

                y_sb = ypool.tile([H3, sw], f32, tag="y", name=f"y_{sup}",
                                  padded_shape=[H3, SUP])
                for pb in range(sw // NB):
                    h1s = []
                    for mc in range(NMC):
                        p1 = ps1p.tile([MC, NB], f32, tag="l1",
                                       name=f"p1_{sup}_{pb}_{mc}")
                        for kt in range(NKT):
                            nc.tensor.matmul(
                                p1[:],
                                w1eff_t[kt][:, mc * MC:(mc + 1) * MC],
                                xtile[:, kt * sw + pb * NB:
                                      kt * sw + (pb + 1) * NB],
                                start=(kt == 0),
                                stop=(kt == NKT - 1),
                            )
                        h1 = hpool.tile([MC, NB], bf16, tag=f"h1_{mc}",
                                        name=f"h1_{sup}_{pb}_{mc}")
                        nc.scalar.activation(
                            h1[:], p1[:], relu,
                            bias=bias_sb[:, mc:mc + 1], scale=1.0,
                        )
                        h1s.append(h1)

                    p2 = ps2p.tile([H2, NB], f32, tag="l2",
                                   name=f"p2_{sup}_{pb}")
                    for k2 in range(3):
                        nc.tensor.matmul(
                            p2[:], w2_sb[:, k2 * H2:(k2 + 1) * H2],
                            h1s[k2][:],
                            start=(k2 == 0), stop=(k2 == 2),
                        )
                    h2 = hpool.tile([H2, NB], bf16, tag="h2",
                                    name=f"h2_{sup}_{pb}")
                    nc.vector.tensor_scalar(
                        h2[:], p2[:], bias_sb[:, 3:4], 0.0, alu_add, alu_max
                    )

                    p3 = ps3p.tile([H3, NB], f32, tag="l3",
                                   name=f"p3_{sup}_{pb}")
                    nc.tensor.matmul(p3[:], w3_sb[:], h2[:],
                                     start=True, stop=True)
                    nc.vector.tensor_scalar_add(
                        y_sb[:, pb * NB:(pb + 1) * NB], p3[:],
                        bias_sb[:H3, 4:5])

                nc.sync.dma_start(yT_d[:, sup_start:sup_start + sw], y_sb[:])
                sup_start += sw

    nc.compile()
    return nc


def _build_amat_banded(conv_w: np.ndarray) -> np.ndarray:
    """Scatter the 9 conv weights into the banded A^T [QP, ABW]:
    A^T[q, u] = conv_w[ki, kj] for q = 26*i + j, u = 28*(i+ki) + (j+kj),
    stored per 128-row q-tile with columns [c_lo*KT, c_hi*KT) of the band."""
    amat = np.zeros((Q, U), np.float32)
    i = np.arange(26)
    j = np.arange(26)
    q = (26 * i[:, None] + j[None, :]).ravel()
    for ki in range(3):
        for kj in range(3):
            u = (28 * (i[:, None] + ki) + j[None, :] + kj).ravel()
            amat[q, u] = conv_w[ki, kj]
    banded = np.zeros((QP, ABW), np.float32)
    for (q0, p_real, c_lo, c_hi) in _fold_bands():
        w = (c_hi - c_lo) * KT
        banded[q0:q0 + p_real, :w] = amat[q0:q0 + p_real, c_lo * KT:c_hi * KT]
    return banded


def _make_in_maps(x, conv_w, W1, b1, W2, b2, W3, b3):
    bf = ml_dtypes.bfloat16
    xT = np.ascontiguousarray(x.T.astype(bf))  # [U, B] bf16
    foldpk = np.zeros((QP, ABW + H1), np.float32)
    foldpk[:, :ABW] = _build_amat_banded(conv_w)
    foldpk[:Q, ABW:] = np.asarray(W1, np.float32)
    foldpk = np.ascontiguousarray(foldpk.astype(bf))
    w2 = np.ascontiguousarray(np.asarray(W2, np.float32).astype(bf))
    w3 = np.ascontiguousarray(np.asarray(W3, np.float32).astype(bf))
    bias = np.zeros((MC, 5), np.float32)
    bias[:, :NMC] = np.asarray(b1, np.float32).reshape(NMC, MC).T
    bias[:, 3] = np.asarray(b2, np.float32)
    bias[:H3, 4] = np.asarray(b3, np.float32)
    in_maps = []
    for c in range(N_CORES):
        in_maps.append({
            "xT": np.ascontiguousarray(xT[:, c * BC:(c + 1) * BC]),
            "fold": foldpk,
            "w2": w2, "w3": w3,
            "bias": bias,
        })
    return in_maps


def kernel(x, conv_w, W1, b1, W2, b2, W3, b3):
    x = np.asarray(x, dtype=np.float32)
    conv_w = np.asarray(conv_w, dtype=np.float32)

    if "nc" not in _prog_cache:
        _prog_cache["nc"] = _build_program()
    nc = _prog_cache["nc"]

    in_maps = _make_in_maps(x, conv_w, W1, b1, W2, b2, W3, b3)
    res = bass_utils.run_bass_kernel_spmd(
        nc, in_maps, core_ids=list(range(N_CORES))
    )

    out = np.empty((B, H3), np.float32)
    for c in range(N_CORES):
        out[c * BC:(c + 1) * BC, :] = res.results[c]["yT"].T
    return out


# revision 20
# speedup vs baseline: 1.2073x; 1.2073x over previous
"""Trainium2 Bass kernel for the digit-conv model.

Math: y = relu(relu(conv3x3(x) @ W1 + b1) @ W2 + b2) @ W3 + b3.
The valid 3x3 conv is linear, so it folds into W1 on device:
feat = x @ A with A[u, q] sparse from conv_w, hence
W1eff = A @ W1 and y = mlp(x @ W1eff ...). The kernel computes
W1eff = A^T.T @ W1 on the tensor engine once (A^T is banded, so
all-zero blocks are statically skipped), then streams the batch
through the 3-layer MLP entirely as lhsT.T @ rhs matmuls with channels
on partitions and batch on the free dimension (no transposes needed:
x is supplied pre-transposed per shard, and every weight is already in
[K, M] layout).

Sharding: pure data parallel — batch split across 8 cores, weights
replicated. Host-side work is limited to layout (x transpose + shard +
bf16 cast, zero-padding, band extraction) and scattering the 9 conv
weights into the A^T matrix (no arithmetic).

All matmul operands are bf16: fp32 x (25.7 MB/core) would be DMA-bound
(~250 GB/s/core effective -> ~103 us, above the ~97 us tensor-engine
floor). PSUM accumulation is fp32; biases are applied in fp32 from
PSUM. (f32r was measured more accurate but cannot be mixed with bf16:
walrus inserts round-to-fp32r passes over DMA-loaded f32r tiles that
corrupt neighboring tiles in a mixed-dtype program.)

DMA instruction count is minimized (one multi-tile DMA per logical
group via 3D access patterns): each dma_start costs ~650 ns of serial
issue on its queue engine, which dominated startup in earlier
revisions. Constants go through the GpSimd (SWDGE) path so x loads
start issuing immediately on the Sync (HWDGE) path. A block of dummy
matmuls on a memset tile warms the PE clock-gate (HAM) before real
work arrives, and the first batch super-blocks are narrower so the
main loop starts after ~1.3 MB of x instead of ~3.2 MB.
"""

import ml_dtypes
import numpy as np

import concourse.tile as tile
from concourse import bacc, mybir
from concourse import bass_utils

N_CORES = 8
B = 65536
BC = B // N_CORES  # 8192 rows per core
U = 784            # input features (28*28)
Q = 676            # conv outputs (26*26)
QP = 768           # q padded to 6 full tiles of 128
H1, H2, H3 = 300, 100, 10
NB = 512           # batch columns per PSUM block (one bank of fp32)
SUP = 2048         # max batch columns per DMA super-block
SUP_WIDTHS = [512, 512, 1024, 2048, 2048, 1536, 512]
assert sum(SUP_WIDTHS) == BC
KT = 112           # u-dim k-tile (784 = 7*112)
NKT = 7
MC = 100           # layer-1 output chunk (300 = 3*100)
NMC = 3
ABW = 336          # amat band width (3 u-chunks), fixed for all q-tiles

_prog_cache = {}


def _fold_bands():
    """Static block-sparsity of A^T [Q, U]: per 128-row q-tile, the nonzero
    columns lie in a band; returns per-tile (q0, p_real, c_lo, c_hi) with the
    band given in whole 112-wide u-chunks (at most 3 chunks wide)."""
    bands = []
    for qt in range(QP // 128):
        q0 = qt * 128
        p_real = min(128, Q - q0)
        i_lo = q0 // 26
        i_hi = (q0 + p_real - 1) // 26
        u_lo = 28 * i_lo
        u_hi = min(U, 28 * (i_hi + 2) + 28)   # exclusive upper bound
        c_lo = u_lo // KT
        c_hi = (u_hi + KT - 1) // KT          # exclusive chunk bound
        assert c_hi - c_lo <= ABW // KT
        bands.append((q0, p_real, c_lo, c_hi))
    return bands


def _build_program():
    f32 = mybir.dt.float32
    bf16 = mybir.dt.bfloat16
    relu = mybir.ActivationFunctionType.Relu
    alu_add = mybir.AluOpType.add
    alu_max = mybir.AluOpType.max

    nc = bacc.Bacc(
        "TRN2", target_bir_lowering=False, debug=False, num_devices=N_CORES
    )

    nqt = QP // 128
    xT_d = nc.dram_tensor("xT", [U, BC], bf16, kind="ExternalInput").ap()
    FW = ABW + H1  # 636: packed [amat band | w1] row width
    fold_d = nc.dram_tensor("fold", [QP, FW], bf16, kind="ExternalInput").ap()
    w2_d = nc.dram_tensor("w2", [H1, H2], bf16, kind="ExternalInput").ap()
    w3_d = nc.dram_tensor("w3", [H2, H3], bf16, kind="ExternalInput").ap()
    bias_d = nc.dram_tensor("bias", [MC, 5], f32, kind="ExternalInput").ap()
    yT_d = nc.dram_tensor("yT", [H3, BC], f32, kind="ExternalOutput").ap()

    bands = _fold_bands()

    with tile.TileContext(nc) as tc:
        with tc.tile_pool(name="const", bufs=1) as cpool, \
             tc.tile_pool(name="xp", bufs=4) as xpool, \
             tc.tile_pool(name="hp", bufs=2) as hpool, \
             tc.tile_pool(name="yp", bufs=2) as ypool, \
             tc.tile_pool(name="ps1", bufs=5, space="PSUM") as ps1p, \
             tc.tile_pool(name="ps2", bufs=2, space="PSUM") as ps2p, \
             tc.tile_pool(name="ps3", bufs=1, space="PSUM") as ps3p:

            # ---- HAM warmup: dummy matmuls on a memset tile so the PE
            # clock-gate releases before the real work arrives ----
            warm_sb = cpool.tile([128, 128], bf16)
            nc.vector.memset(warm_sb[:], 0.0)
            for wi in range(44):
                pw = ps2p.tile([128, 64], f32, tag="l2", name=f"pwarm_{wi}")
                nc.tensor.matmul(pw[:], warm_sb[:, :128], warm_sb[:, :64],
                                 start=True, stop=True)

            # ---- constants into SBUF (one merged DMA per group, on the
            # SWDGE path so they don't block x-load issue on HWDGE) ----
            fold_sb = cpool.tile([128, nqt * FW], bf16)
            nc.sync.dma_start(
                fold_sb[:].rearrange("p (q c) -> p q c", c=FW),
                fold_d.rearrange("(q p) c -> p q c", p=128),
            )
            w2_sb = cpool.tile([MC, NMC * H2], bf16)
            nc.gpsimd.dma_start(
                w2_sb[:].rearrange("p (k c) -> p k c", c=H2),
                w2_d.rearrange("(k p) c -> p k c", p=MC),
            )
            w3_sb = cpool.tile([H2, H3], bf16)
            nc.gpsimd.dma_start(w3_sb[:], w3_d)
            bias_sb = cpool.tile([MC, 5], f32)
            nc.gpsimd.dma_start(bias_sb[:], bias_d)

            # ---- fold the conv into W1: W1eff[u, c] = (A^T).T @ W1 ----
            # Only q-tiles whose band covers the u-chunk contribute; the
            # rest are all-zero blocks of the banded A^T and are skipped.
            # (fold PSUM shares the l1 slot group: same tag, bank-sized)
            w1eff_t = []
            for ut in range(NKT):
                parts = [qt for qt, (_, _, c_lo, c_hi) in enumerate(bands)
                         if c_lo <= ut < c_hi]
                assert parts
                pf = ps1p.tile([KT, NB], f32, tag="l1", name=f"pfold_{ut}")
                for idx, qt in enumerate(parts):
                    _, _, c_lo, _ = bands[qt]
                    off = qt * FW + (ut - c_lo) * KT
                    nc.tensor.matmul(
                        pf[:, :H1],
                        fold_sb[:, off:off + KT],
                        fold_sb[:, qt * FW + ABW:(qt + 1) * FW],
                        start=(idx == 0),
                        stop=(idx == len(parts) - 1),
                    )
                we = cpool.tile([KT, H1], bf16, name=f"w1eff_{ut}")
                nc.vector.tensor_copy(we[:], pf[:, :H1])
                w1eff_t.append(we)

            # ---- main pipeline over batch super-blocks ----
            sup_start = 0
            for sup, sw in enumerate(SUP_WIDTHS):
                xtile = xpool.tile([KT, NKT * sw], bf16, tag="x",
                                   name=f"xt_{sup}",
                                   padded_shape=[KT, NKT * SUP])
                nc.sync.dma_start(
                    xtile[:].rearrange("p (k c) -> p k c", c=sw),
                    xT_d[:, sup_start:sup_start + sw]
                    .rearrange("(k p) c -> p k c", p=KT),
                )

                y_sb = ypool.tile([H3, sw], f32, tag="y", name=f"y_{sup}",
                                  padded_shape=[H3, SUP])
                for pb in range(sw // NB):
                    h1s = []
                    for mc in range(NMC):
                        p1 = ps1p.tile([MC, NB], f32, tag="l1",
                                       name=f"p1_{sup}_{pb}_{mc}")
                        for kt in range(NKT):
                            nc.tensor.matmul(
                                p1[:],
                                w1eff_t[kt][:, mc * MC:(mc + 1) * MC],
                                xtile[:, kt * sw + pb * NB:
                                      kt * sw + (pb + 1) * NB],
                                start=(kt == 0),
                                stop=(kt == NKT - 1),
                            )
                        h1 = hpool.tile([MC, NB], bf16, tag=f"h1_{mc}",
                                        name=f"h1_{sup}_{pb}_{mc}")
                        nc.scalar.activation(
                            h1[:], p1[:], relu,
                            bias=bias_sb[:, mc:mc + 1], scale=1.0,
                        )
                        h1s.append(h1)

                    p2 = ps2p.tile([H2, NB], f32, tag="l2",
                                   name=f"p2_{sup}_{pb}")
                    for k2 in range(3):
                        nc.tensor.matmul(
                            p2[:], w2_sb[:, k2 * H2:(k2 + 1) * H2],
                            h1s[k2][:],
                            start=(k2 == 0), stop=(k2 == 2),
                        )
                    h2 = hpool.tile([H2, NB], bf16, tag="h2",
                                    name=f"h2_{sup}_{pb}")
                    nc.vector.tensor_scalar(
                        h2[:], p2[:], bias_sb[:, 3:4], 0.0, alu_add, alu_max
                    )

                    p3 = ps3p.tile([H3, NB], f32, tag="l3",
                                   name=f"p3_{sup}_{pb}")
                    nc.tensor.matmul(p3[:], w3_sb[:], h2[:],
                                     start=True, stop=True)
                    nc.vector.tensor_scalar_add(
                        y_sb[:, pb * NB:(pb + 1) * NB], p3[:],
                        bias_sb[:H3, 4:5])

                nc.sync.dma_start(yT_d[:, sup_start:sup_start + sw], y_sb[:])
                sup_start += sw

    nc.compile()
    return nc


def _build_amat_banded(conv_w: np.ndarray) -> np.ndarray:
    """Scatter the 9 conv weights into the banded A^T [QP, ABW]:
    A^T[q, u] = conv_w[ki, kj] for q = 26*i + j, u = 28*(i+ki) + (j+kj),
    stored per 128-row q-tile with columns [c_lo*KT, c_hi*KT) of the band."""
    amat = np.zeros((Q, U), np.float32)
    i = np.arange(26)
    j = np.arange(26)
    q = (26 * i[:, None] + j[None, :]).ravel()
    for ki in range(3):
        for kj in range(3):
            u = (28 * (i[:, None] + ki) + j[None, :] + kj).ravel()
            amat[q, u] = conv_w[ki, kj]
    banded = np.zeros((QP, ABW), np.float32)
    for (q0, p_real, c_lo, c_hi) in _fold_bands():
        w = (c_hi - c_lo) * KT
        banded[q0:q0 + p_real, :w] = amat[q0:q0 + p_real, c_lo * KT:c_hi * KT]
    return banded


def _make_in_maps(x, conv_w, W1, b1, W2, b2, W3, b3):
    bf = ml_dtypes.bfloat16
    xT = np.ascontiguousarray(x.T.astype(bf))  # [U, B] bf16
    foldpk = np.zeros((QP, ABW + H1), np.float32)
    foldpk[:, :ABW] = _build_amat_banded(conv_w)
    foldpk[:Q, ABW:] = np.asarray(W1, np.float32)
    foldpk = np.ascontiguousarray(foldpk.astype(bf))
    w2 = np.ascontiguousarray(np.asarray(W2, np.float32).astype(bf))
    w3 = np.ascontiguousarray(np.asarray(W3, np.float32).astype(bf))
    bias = np.zeros((MC, 5), np.float32)
    bias[:, :NMC] = np.asarray(b1, np.float32).reshape(NMC, MC).T
    bias[:, 3] = np.asarray(b2, np.float32)
    bias[:H3, 4] = np.asarray(b3, np.float32)
    in_maps = []
    for c in range(N_CORES):
        in_maps.append({
            "xT": np.ascontiguousarray(xT[:, c * BC:(c + 1) * BC]),
            "fold": foldpk,
            "w2": w2, "w3": w3,
            "bias": bias,
        })
    return in_maps


def kernel(x, conv_w, W1, b1, W2, b2, W3, b3):
    x = np.asarray(x, dtype=np.float32)
    conv_w = np.asarray(conv_w, dtype=np.float32)

    if "nc" not in _prog_cache:
        _prog_cache["nc"] = _build_program()
    nc = _prog_cache["nc"]

    in_maps = _make_in_maps(x, conv_w, W1, b1, W2, b2, W3, b3)
    res = bass_utils.run_bass_kernel_spmd(
        nc, in_maps, core_ids=list(range(N_CORES))
    )

    out = np.empty((B, H3), np.float32)
    for c in range(N_CORES):
        out[c * BC:(c + 1) * BC, :] = res.results[c]["yT"].T
    return out
